# revision 15
# baseline (speedup 1.0000x reference)
"""Causal self-attention (B=2, T=2048, C=1024, H=16) on 8 Trainium2 NeuronCores.

Sharding: data-parallel over batch (2) x tensor-parallel over heads (4 groups
of 4 heads) = 8 cores. c_attn column-sharded, c_proj row-sharded; each core
emits a partial [C, T] bf16 projection output that the host sums per batch.

All matmuls run in bf16 with fp32 PSUM accumulation. Attention scores are
computed transposed (S^T = K Q^T, k on partitions) as row-tiled concurrent
pairs (two heads on PE row groups h0/h64). The PV matmul keeps V stationary
(65 columns: 64 V dims + a ones column that accumulates the softmax
denominator) and is interleaved into the SAME phase as its S strip: each
S k-tile step immediately feeds the two heads' psy accumulation matmuls
(with a small emission lag so the previous strip's psum frees in time).
This keeps the PE self-sufficient during the exp-bound late strips.

The softmax normalization row 1/denom is broadcast across partitions with a
gpsimd partition_broadcast mid-kernel; the last strip's chains instead use a
K=1 PE broadcast matmul + scalar copy (no gpsimd launch lag) so the final
projection is not gated behind a long serial chain.

DMA strategy: the host repacks x (strip-major), wqk (mt-block-major), wv and
wp so that every input lands with one large DMA of 2-8KB contiguous lines,
split across both HWDGE queues, ordered by first use. Dummy warm-up matmuls
on a vector-memset scratch tile run during the DMA prologue so the PE HAM
clock-gate is already released when real work starts; a few more dummies in
the late phases stop the HAM from re-throttling while the PE waits on exps.
"""

import numpy as np
import ml_dtypes

BF = ml_dtypes.bfloat16

B, T, C, H, DH = 2, 2048, 1024, 16, 64
N_CORES = 8
G = 4            # head groups (tensor-parallel)
HPG = 4          # heads per group
TQ = 512         # query strip width
TK = 128         # key tile width
NSTRIP = T // TQ        # 4 query strips
NKT = T // TK           # 16 key tiles
NCT = C // 128          # 8 contraction tiles for qkv
VST = 136               # V2 per-k-tile stride: 2 heads x (64 V + 1 ones + 3 pad)

_CACHE = {}


def _ensure_runtime():
    """Import jax (boots the axon PJRT plugin) exactly once."""
    import jax
    jax.devices()


def _build(with_bias: bool):
    import concourse.tile as tile
    from concourse import bacc, mybir

    f32 = mybir.dt.float32
    bf16 = mybir.dt.bfloat16
    Exp = mybir.ActivationFunctionType.Exp
    Ln = mybir.ActivationFunctionType.Ln

    nc = bacc.Bacc("TRN2", target_bir_lowering=False, debug=False,
                   enable_asserts=False, num_devices=N_CORES)

    # host-repacked layouts (see _shard_inputs):
    #   xT:  [4 strips * 128 p, 8 ci * 512 col]   (8KB lines)
    #   wqk: [4 mt * 128 p, 8 ci * 128 col]       (2KB lines)
    #   wv:  [128 p, 8 ci * 256 col]              (4KB lines)
    #   wp:  [128 p, 2 pair * 1024 col]           (4KB lines)
    xT_d = nc.dram_tensor("xT", [NSTRIP * 128, NCT * TQ], bf16,
                          kind="ExternalInput").ap()
    wqk_d = nc.dram_tensor("wqk", [4 * 128, NCT * 128], bf16,
                           kind="ExternalInput").ap()
    wv_d = nc.dram_tensor("wv", [128, NCT * 256], bf16, kind="ExternalInput").ap()
    wp_d = nc.dram_tensor("wp", [128, 2 * C], bf16, kind="ExternalInput").ap()
    if with_bias:
        bqk_d = nc.dram_tensor("bqk", [1, 512], bf16, kind="ExternalInput").ap()
        bv_d = nc.dram_tensor("bv", [1, 256], bf16, kind="ExternalInput").ap()
    out_d = nc.dram_tensor("outT", [C, T], bf16, kind="ExternalOutput").ap()

    with tile.TileContext(nc) as tc:
        with (
            tc.tile_pool(name="persist", bufs=1) as pp,
            tc.tile_pool(name="pP", bufs=10) as pP,
            tc.tile_pool(name="rrow", bufs=6) as pRR,
            tc.tile_pool(name="bcsb", bufs=6) as pBC,
            tc.tile_pool(name="ob", bufs=4) as pO,
            tc.tile_pool(name="psum", bufs=1, space="PSUM") as psp,
        ):
            # ---- persistent SBUF tensors (single tiles => single DMAs) --
            # xA strip-major: [p, strip, ci, col]
            xA = pp.tile([128, NSTRIP * NCT * TQ], bf16, tag="xA", name="xA")
            # wqkA mt-major: [p, mt, ci, col]
            wqkA = pp.tile([128, 4 * NCT * 128], bf16, tag="wqkA", name="wqkA")
            wvA = pp.tile([128, NCT * 256], bf16, tag="wvA", name="wvA")
            wp = pp.tile([128, 2 * C], bf16, tag="wp", name="wp")
            QTp = [pp.tile([128, T], bf16, tag=f"QT{p}", name=f"QT{p}")
                   for p in range(2)]
            KTp = [pp.tile([128, T], bf16, tag=f"KT{p}", name=f"KT{p}")
                   for p in range(2)]
            V2 = pp.tile([128, 2 * NKT * VST], bf16, tag="V2", name="V2")
            yT2 = [pp.tile([128, T], bf16, tag=f"yT{p}", name=f"yT{p}")
                   for p in range(2)]
            tri2 = pp.tile([128, 256], bf16, tag="tri2", name="tri2")
            ones64f = pp.tile([1, 64], f32, tag="ones64f", name="ones64f")
            wtile = pp.tile([128, 128], bf16, tag="wtile", name="wtile")
            if with_bias:
                bqk = pp.tile([1, 512], bf16, tag="bqk", name="bqk")
                bv = pp.tile([1, 256], bf16, tag="bv", name="bv")
                ones_row = pp.tile([1, 512], bf16, tag="ones", name="ones")

            def xa(ci, strip):
                base = (strip * NCT + ci) * TQ
                return xA[:, base:base + TQ]

            def wqka(ci, mt):
                base = (mt * NCT + ci) * 128
                return wqkA[:, base:base + 128]

            def wva(ci):
                return wvA[:, ci * 256:(ci + 1) * 256]

            # ---- input DMAs: few, large lines, ordered by first use ----
            def x_strip_dma(eng, strip):
                eng.dma_start(
                    xA[:, strip * NCT * TQ:(strip + 1) * NCT * TQ],
                    xT_d[strip * 128:(strip + 1) * 128, :])

            def wqk_mt_dma(eng, mt):
                eng.dma_start(
                    wqkA[:, mt * NCT * 128:(mt + 1) * NCT * 128],
                    wqk_d[mt * 128:(mt + 1) * 128, :])

            # P0 needs x strip 0 + wqk mt 0/2 (qk) + wv (v steps 0-3).
            x_strip_dma(nc.sync, 0)
            wqk_mt_dma(nc.scalar, 0)
            nc.scalar.dma_start(wvA[:], wv_d[:, :])
            wqk_mt_dma(nc.scalar, 2)
            x_strip_dma(nc.sync, 1)
            wqk_mt_dma(nc.scalar, 1)
            wqk_mt_dma(nc.scalar, 3)
            x_strip_dma(nc.sync, 2)
            x_strip_dma(nc.sync, 3)
            nc.scalar.dma_start(wp[:], wp_d[:, :])

            # ---- PE warm-up during the DMA prologue --------------------
            # the scratch tile is memset by the (idle) vector engine so the
            # warm-ups have no gpsimd dependency and start immediately.
            nc.vector.memset(wtile[:], 0.015625)
            warm = psp.tile([128, 128], f32, tag="S", bufs=2, name="warm")
            for _ in range(46):
                nc.tensor.matmul(warm[:], lhsT=wtile[:], rhs=wtile[:],
                                 start=True, stop=True)

            def keep_warm(n=1, cols=128):
                """Cheap dummy matmuls to keep the PE HAM clock-gate open.
                They write (a prefix of) the current 'S'-tag psum slot, so
                their WAR deps mirror the real S steps' and never add waits
                beyond what the next S matmul already has."""
                wps = psp.tile([128, cols], f32, tag="S", bufs=2, name="warm2")
                for _ in range(n):
                    nc.tensor.matmul(wps[:], lhsT=wtile[:], rhs=wtile[:, 0:cols],
                                     start=True, stop=True)

            # ---- constants ---------------------------------------------
            # tri2[i, 256] = two side-by-side copies of tri[i, j] = (i <= j)
            nc.gpsimd.memset(tri2[:], 1.0)
            for hh in range(2):
                nc.gpsimd.affine_select(
                    out=tri2[:, hh * 128:(hh + 1) * 128],
                    in_=tri2[:, hh * 128:(hh + 1) * 128],
                    compare_op=mybir.AluOpType.is_ge,
                    fill=0.0, base=0, pattern=[[1, 128]], channel_multiplier=-1)
            if with_bias:
                nc.sync.dma_start(bqk[:], bqk_d[:, :])
                nc.sync.dma_start(bv[:], bv_d[:, :])
                nc.gpsimd.memset(ones_row[:], 1.0)
            # ones columns in V2 (denominator accumulators), one strided memset
            v2v = V2[:].rearrange("q (p kt h e) -> q p kt h e", p=2, kt=NKT, h=2)
            nc.gpsimd.memset(v2v[:, :, :, :, 64:65], 1.0)
            nc.gpsimd.memset(ones64f[:], 1.0)

            nbias = 1 if with_bias else 0

            # ---- QKV projection steps ----------------------------------
            def qk_step(pair, mt, qt):
                def f():
                    ps = psp.tile([128, TQ], f32, tag="big", bufs=2, name="psA")
                    for ci in range(NCT):
                        nc.tensor.matmul(
                            ps[:],
                            lhsT=wqka(ci, mt),
                            rhs=xa(ci, qt),
                            start=(ci == 0), stop=(ci == NCT + nbias - 1))
                    if with_bias:
                        nc.tensor.matmul(
                            ps[:], lhsT=bqk[0:1, mt * 128:(mt + 1) * 128],
                            rhs=ones_row[0:1, 0:TQ], start=False, stop=True)
                    dst = QTp[pair] if mt < 2 else KTp[pair]
                    nc.vector.tensor_copy(dst[:, qt * TQ:(qt + 1) * TQ], ps[:])
                return f

            def v_step(kt):
                def f():
                    psv = psp.tile([128, 256], f32, tag="big", bufs=2, name="psVt")
                    strip, sub = kt // 4, kt % 4
                    for ci in range(NCT):
                        nc.tensor.matmul(
                            psv[:],
                            lhsT=xa(ci, strip)[:, sub * 128:(sub + 1) * 128],
                            rhs=wva(ci),
                            start=(ci == 0), stop=(ci == NCT + nbias - 1))
                    if with_bias:
                        nc.tensor.matmul(
                            psv[:], lhsT=ones_row[0:1, 0:128], rhs=bv[0:1, :],
                            start=False, stop=True)
                    # [128, 2p, 2h, 64] -> strided V2 slots in one copy
                    s4 = psv[:].rearrange("q (p h b) -> q p h b", p=2, h=2)
                    d4 = v2v[:, :, kt, :, 0:64]
                    nc.vector.tensor_copy(d4, s4)
                return f

            def A_qk(pair, qt):
                return [qk_step(pair, mt, qt) for mt in (pair, 2 + pair)]

            def A_v(qt):
                return [v_step(kt) for kt in range(4 * qt, 4 * qt + 4)]

            # ---- softmax normalization chains --------------------------
            def chain_step(pair, qt, hh, psy):
                """1/denominator, broadcast over 64 partitions, times psy:
                transpose the denominator row to partitions with a DMA
                round-trip, bf16 reciprocal, gpsimd partition_broadcast."""
                def f():
                    ph = psy[hh]
                    drow = pRR.tile([1, TQ], f32, tag="rr", name="drow")
                    nc.vector.tensor_copy(drow[:], ph[64:65, :])
                    rec4 = pRR.tile([128, 4], f32, tag="r4", name="rec4")
                    nc.sync.dma_start(rec4[:, :], drow[0:1, :])
                    rec4b = pRR.tile([128, 4], bf16, tag="r4b", name="rec4b")
                    with nc.allow_low_precision("softmax recip in bf16"):
                        nc.vector.reciprocal(rec4b[:], rec4[:])
                    rrow = pRR.tile([1, TQ], bf16, tag="rrb", name="rrow")
                    nc.sync.dma_start(rrow[0:1, :], rec4b[:, :])
                    bcs = pBC.tile([64, TQ], bf16, tag="bc", name="bcs")
                    nc.gpsimd.partition_broadcast(bcs[:], rrow[0:1, :])
                    nc.vector.tensor_mul(
                        yT2[pair][hh * 64:(hh + 1) * 64, qt * TQ:(qt + 1) * TQ],
                        ph[0:64, :], bcs[:])
                return f

            # ---- fused S + PV strip ------------------------------------
            # Each k-tile step: row-tiled concurrent S matmul pair (the two
            # heads on PE rows 0-63 / 64-127), one exp, and - lagged by a few
            # steps so the previous strip's psum frees - the two heads' psy
            # accumulation matmuls. Chains are returned separately.
            def strip_steps(pair, qt, warm_every=0):
                nk = 4 * (qt + 1)
                P_store = {}
                psy = [None, None]
                LAG = 5

                def s_step(kt):
                    def f():
                        ps = psp.tile([128, 2 * TQ], f32, tag="S", bufs=2, name="psS")
                        m = kt - 4 * qt
                        off = max(0, m) * 128
                        for hh in range(2):
                            nc.tensor.matmul(
                                ps[:, hh * TQ + off:(hh + 1) * TQ],
                                lhsT=KTp[pair][hh * 64:(hh + 1) * 64,
                                               kt * 128:(kt + 1) * 128],
                                rhs=QTp[pair][hh * 64:(hh + 1) * 64,
                                              qt * TQ + off:(qt + 1) * TQ],
                                start=True, stop=True)
                        Pt = pP.tile([128, 2 * TQ], bf16, tag="P", bufs=10, name="Pt")
                        if m < 0:    # one exp across both heads' banks
                            nc.scalar.activation(Pt[:, :], ps[:, :], Exp, scale=0.125)
                        else:        # one strided exp covering both heads
                            pse = ps[:].rearrange("p (h w) -> p h w", h=2)[:, :, off:TQ]
                            pte = Pt[:].rearrange("p (h w) -> p h w", h=2)[:, :, off:TQ]
                            nc.scalar.activation(pte, pse, Exp, scale=0.125)
                            # diagonal block: keep k <= q only, both heads at once
                            ptm = Pt[:].rearrange("p (h w) -> p h w", h=2)[:, :, off:off + 128]
                            nc.vector.tensor_mul(
                                ptm, ptm,
                                tri2[:].rearrange("p (h w) -> p h w", h=2))
                        P_store[kt] = Pt
                    return f

                def psy_step(kt):
                    def f():
                        off = max(0, kt - 4 * qt) * 128
                        if kt == 0:
                            for hh in range(2):
                                psy[hh] = psp.tile([65, TQ], f32, tag="pv",
                                                   bufs=2, name="psy")
                        Pt = P_store.pop(kt)
                        for hh in range(2):
                            nc.tensor.matmul(
                                psy[hh][:, off:TQ],
                                lhsT=v2v[:, pair, kt, hh, 0:65],
                                rhs=Pt[:, hh * TQ + off:(hh + 1) * TQ],
                                start=(kt == 0), stop=(kt == nk - 1))
                    return f

                steps = []
                emitted = 0
                for kt in range(nk):
                    steps.append(s_step(kt))
                    if warm_every and kt % warm_every == warm_every - 1:
                        steps.append(lambda: keep_warm(1))
                    backlog = min(LAG, nk - 1 - kt)
                    while kt + 1 - emitted > backlog:
                        steps.append(psy_step(emitted))
                        emitted += 1
                chains = [chain_step(pair, qt, hh, psy) for hh in range(2)]
                return steps, chains, psy

            # ---- output projection -------------------------------------
            def co_mm(pso, pair, co, qt):
                nc.tensor.matmul(
                    pso[:],
                    lhsT=wp[:, pair * C + co * 128:pair * C + (co + 1) * 128],
                    rhs=yT2[pair][:, qt * TQ:(qt + 1) * TQ],
                    start=(pair == 0), stop=(pair == 1))

            def co_out(pso, co, qt, eng=None, scopy=False):
                ob = pO.tile([128, TQ], bf16, tag="ob", name="ob")
                if scopy:
                    nc.scalar.copy(ob[:], pso[:])
                else:
                    nc.vector.tensor_copy(ob[:], pso[:])
                (eng or nc.sync).dma_start(
                    out_d[co * 128:(co + 1) * 128, qt * TQ:(qt + 1) * TQ],
                    ob[:])

            def PJ_steps(qt):
                def co_step(co):
                    def f():
                        pso = psp.tile([128, TQ], f32, tag="big", bufs=2, name="psO")
                        for pair in range(2):
                            co_mm(pso, pair, co, qt)
                        co_out(pso, co, qt)
                    return f

                return [co_step(co) for co in range(8)]

            def PJ_final(qt, psy):
                """Final-strip projection, interleaved with the last two
                normalization chains. Those chains avoid DMA round-trips and
                the gpsimd launch lag entirely: 1/d = exp(-ln d), with ln(d)
                taken straight off the psum denominator row, broadcast over
                64 partitions by a K=1 fp32 PE matmul, and the exp(-x)
                fused into the activation that evacuates the broadcast.
                PE instructions are emitted in expected-readiness order
                (the engine queue is strict FIFO - a blocked matmul stalls
                everything behind it). Output DMAs alternate HWDGE queues."""
                pend = {}
                lnrow = [None, None]
                bcp = [None, None]

                def half_mm(pso, co, hh, start, stop):
                    sl = slice(hh * 64, (hh + 1) * 64)
                    nc.tensor.matmul(
                        pso[:],
                        lhsT=wp[sl, C + co * 128:C + (co + 1) * 128],
                        rhs=yT2[1][sl, qt * TQ:(qt + 1) * TQ],
                        start=start, stop=stop)

                def ln_pre(hh):
                    def f():
                        lnrow[hh] = pRR.tile([1, TQ], f32, tag="lnr",
                                             name="lnrow")
                        nc.scalar.activation(lnrow[hh][:], psy[hh][64:65, :],
                                             Ln)
                    return f

                def ln_bcast(hh):
                    def f():
                        bcp[hh] = psp.tile([64, TQ], f32, tag="S", bufs=2,
                                           name="bcp")
                        nc.tensor.matmul(bcp[hh][:], lhsT=ones64f[:],
                                         rhs=lnrow[hh][:], start=True,
                                         stop=True)
                    return f

                def ln_post(hh):
                    def f():
                        bcs = pBC.tile([64, TQ], bf16, tag="bc", name="bcs")
                        with nc.allow_low_precision("softmax recip via exp-ln"):
                            nc.scalar.activation(bcs[:], bcp[hh][:], Exp,
                                                 scale=-1.0)
                        nc.vector.tensor_mul(
                            yT2[1][hh * 64:(hh + 1) * 64,
                                   qt * TQ:(qt + 1) * TQ],
                            psy[hh][0:64, :], bcs[:])
                    return f

                def open_step(co, tag):
                    def f():
                        pso = psp.tile([128, TQ], f32, tag=tag, bufs=2, name="psO")
                        co_mm(pso, 0, co, qt)
                        pend[co] = pso
                    return f

                def half0_step(co):
                    def f():
                        half_mm(pend[co], co, 0, start=False, stop=False)
                    return f

                def close_step(co):
                    def f():
                        pso = pend.pop(co)
                        half_mm(pso, co, 1, start=False, stop=True)
                        co_out(pso, co, qt, nc.scalar if co % 2 else nc.sync,
                               scopy=bool(co % 2))
                    return f

                def full_step(co):
                    def f():
                        pso = psp.tile([128, TQ], f32, tag="big", bufs=2, name="psO")
                        for pair in range(2):
                            co_mm(pso, pair, co, qt)
                        co_out(pso, co, qt, nc.scalar if co % 2 else nc.sync,
                               scopy=bool(co % 2))
                    return f

                seq = [ln_pre(0), ln_pre(1),
                       open_step(0, "big"), open_step(1, "big"),
                       lambda: keep_warm(3),
                       ln_bcast(0), ln_bcast(1),
                       ln_post(0),
                       lambda: keep_warm(2),
                       ln_post(1),
                       half0_step(0), half0_step(1),
                       open_step(2, "pv"), open_step(3, "pv"),
                       half0_step(2), half0_step(3),
                       close_step(0), close_step(1),
                       close_step(2), close_step(3),
                       full_step(4), full_step(5),
                       full_step(6), full_step(7)]
                return seq

            def weave(s_list, others):
                """Interleave `others` proportionally between strip steps."""
                if not s_list:
                    for f in others:
                        f()
                    return
                r = len(others) / len(s_list)
                acc, oi = 0.5, 0
                for f in s_list:
                    f()
                    acc += r
                    while acc >= 1.0 and oi < len(others):
                        others[oi]()
                        oi += 1
                        acc -= 1.0
                while oi < len(others):
                    others[oi]()
                    oi += 1

            # ---- phase schedule ----------------------------------------
            # strips in order (0,0),(0,1),(1,0),(0,2),(1,1),(0,3),(1,2),(1,3)
            # with QKV step groups placed one phase before their consumers
            # and projections as late fillers.
            for f in A_qk(0, 0) + A_v(0):
                f()

            s00, c00, _ = strip_steps(0, 0)
            s01, c01, _ = strip_steps(0, 1)
            s10, c10, _ = strip_steps(1, 0)
            s02, c02, _ = strip_steps(0, 2)
            s11, c11, _ = strip_steps(1, 1)
            s03, c03, _ = strip_steps(0, 3)
            s12, c12, _ = strip_steps(1, 2, warm_every=4)
            s13, _c13, psy13 = strip_steps(1, 3, warm_every=4)

            pj0 = PJ_steps(0)
            pj1 = PJ_steps(1)
            pj2 = PJ_steps(2)

            weave(s00 + c00, A_qk(0, 1) + A_qk(1, 0) + A_v(1))
            weave(s01 + c01, A_qk(0, 2) + A_v(2))
            weave(s10 + c10, A_qk(1, 1) + A_qk(0, 3))
            weave(s02 + c02, A_v(3) + pj0[:4])
            weave(s11 + c11, A_qk(1, 2) + pj0[4:])
            weave(s03 + c03, A_qk(1, 3) + pj1[:4])
            weave(s12 + c12, pj1[4:])
            weave(s13, pj2)
            weave([], PJ_final(3, psy13))

    nc.compile()
    return nc


def _get_nc(with_bias: bool):
    key = ("nc", with_bias)
    if key not in _CACHE:
        _ensure_runtime()
        _CACHE[key] = _build(with_bias)
    return _CACHE[key]


def _shard_inputs(x, w_qkv, b_qkv, w_proj, with_bias):
    """Build the 8 per-core input maps (bf16), repacked for large DMA lines."""
    in_maps = []
    for core in range(N_CORES):
        b, g = core // G, core % G
        hs = [g * HPG + i for i in range(HPG)]
        q_cols = [w_qkv[:, h * DH:(h + 1) * DH] for h in hs]
        k_cols = [w_qkv[:, C + h * DH: C + (h + 1) * DH] for h in hs]
        v_cols = [w_qkv[:, 2 * C + h * DH: 2 * C + (h + 1) * DH] for h in hs]

        # x: [C, T] -> strip-major [strip*128, ci*512]
        xT = np.ascontiguousarray(x[b].T)              # [C, T]
        xs = xT.reshape(NCT, 128, NSTRIP, TQ)          # [ci, p, strip, col]
        xs = xs.transpose(2, 1, 0, 3).reshape(NSTRIP * 128, NCT * TQ)

        # wqk: [C, 512] (q01 q23 k01 k23) -> mt-major [mt*128, ci*128]
        wqk = np.concatenate(q_cols + k_cols, axis=1)  # [C, 512]
        ws = wqk.reshape(NCT, 128, 4, 128)             # [ci, p, mt, col]
        ws = ws.transpose(2, 1, 0, 3).reshape(4 * 128, NCT * 128)

        # wv: [C, 256] -> [128, ci*256]
        wv = np.concatenate(v_cols, axis=1)            # [C, 256]
        wvs = wv.reshape(NCT, 128, 256).transpose(1, 0, 2).reshape(128, NCT * 256)

        # wp: [256, C] -> [128, pair*1024]
        wpm = np.concatenate(
            [w_proj[h * DH:(h + 1) * DH, :] for h in hs], axis=0)  # [256, C]
        wps = wpm.reshape(2, 128, C).transpose(1, 0, 2).reshape(128, 2 * C)

        m = {
            "xT": np.ascontiguousarray(xs).astype(BF),
            "wqk": np.ascontiguousarray(ws).astype(BF),
            "wv": np.ascontiguousarray(wvs).astype(BF),
            "wp": np.ascontiguousarray(wps).astype(BF),
        }
        if with_bias:
            bq = [b_qkv[h * DH:(h + 1) * DH] for h in hs]
            bk = [b_qkv[C + h * DH: C + (h + 1) * DH] for h in hs]
            bvs = [b_qkv[2 * C + h * DH: 2 * C + (h + 1) * DH] for h in hs]
            m["bqk"] = np.concatenate(bq + bk)[None, :].astype(BF)
            m["bv"] = np.concatenate(bvs)[None, :].astype(BF)
        in_maps.append(m)
    return in_maps


def run_on_device(x, w_qkv, b_qkv, w_proj, b_proj, trace=False, trace_kwargs=None):
    """Returns (output [B,T,C] float32, BassKernelResults)."""
    x = np.asarray(x, np.float32)
    w_qkv = np.asarray(w_qkv, np.float32)
    b_qkv = np.asarray(b_qkv, np.float32)
    w_proj = np.asarray(w_proj, np.float32)
    b_proj = np.asarray(b_proj, np.float32)

    with_bias = bool(np.any(b_qkv))
    nc = _get_nc(with_bias)
    in_maps = _shard_inputs(x, w_qkv, b_qkv, w_proj, with_bias)

    from concourse.bass_utils import run_bass_kernel_spmd
    res = run_bass_kernel_spmd(nc, in_maps, core_ids=list(range(N_CORES)),
                               trace=trace, **(trace_kwargs or {}))

    out = np.zeros((B, T, C), np.float64)
    for core in range(N_CORES):
        b = core // G
        out[b] += res.results[core]["outT"].T.astype(np.float64)
    out += b_proj.astype(np.float64)[None, None, :]
    return out.astype(np.float32), res


def kernel(x, w_qkv, b_qkv, w_proj, b_proj):
    out, _ = run_on_device(x, w_qkv, b_qkv, w_proj, b_proj)
    return out


# revision 21
# speedup vs baseline: 1.0060x; 1.0060x over previous
"""Causal self-attention (B=2, T=2048, C=1024, H=16) on 8 Trainium2 NeuronCores.

Sharding: data-parallel over batch (2) x tensor-parallel over heads (4 groups
of 4 heads) = 8 cores. c_attn column-sharded, c_proj row-sharded; each core
emits a partial [C, T] bf16 projection output that the host sums per batch.

All matmuls run in bf16 with fp32 PSUM accumulation. Attention scores are
computed transposed (S^T = K Q^T, k on partitions) as row-tiled concurrent
pairs (two heads on PE row groups h0/h64). The PV matmul keeps V stationary
(65 columns: 64 V dims + a ones column that accumulates the softmax
denominator) and is interleaved into the SAME phase as its S strip: each
S k-tile step immediately feeds the two heads' psy accumulation matmuls
(with a small emission lag so the previous strip's psum frees in time).
This keeps the PE self-sufficient during the exp-bound late strips.

The softmax normalization row 1/denom is broadcast across partitions with a
gpsimd partition_broadcast mid-kernel; the last strip's chains instead use a
K=1 PE broadcast matmul + scalar copy (no gpsimd launch lag) so the final
projection is not gated behind a long serial chain.

DMA strategy: the host repacks x (strip-major), wqk (mt-block-major), wv and
wp so that every input lands with one large DMA of 2-8KB contiguous lines,
split across both HWDGE queues, ordered by first use. Dummy warm-up matmuls
on a vector-memset scratch tile run during the DMA prologue so the PE HAM
clock-gate is already released when real work starts; a few more dummies in
the late phases stop the HAM from re-throttling while the PE waits on exps.
"""

import numpy as np
import ml_dtypes

BF = ml_dtypes.bfloat16

B, T, C, H, DH = 2, 2048, 1024, 16, 64
N_CORES = 8
G = 4            # head groups (tensor-parallel)
HPG = 4          # heads per group
TQ = 512         # query strip width
TK = 128         # key tile width
NSTRIP = T // TQ        # 4 query strips
NKT = T // TK           # 16 key tiles
NCT = C // 128          # 8 contraction tiles for qkv
VST = 136               # V2 per-k-tile stride: 2 heads x (64 V + 1 ones + 3 pad)

_CACHE = {}


def _ensure_runtime():
    """Import jax (boots the axon PJRT plugin) exactly once."""
    import jax
    jax.devices()


def _build(with_bias: bool):
    import concourse.tile as tile
    from concourse import bacc, mybir

    f32 = mybir.dt.float32
    bf16 = mybir.dt.bfloat16
    Exp = mybir.ActivationFunctionType.Exp
    Ln = mybir.ActivationFunctionType.Ln

    nc = bacc.Bacc("TRN2", target_bir_lowering=False, debug=False,
                   enable_asserts=False, num_devices=N_CORES)

    # host-repacked layouts (see _shard_inputs):
    #   xT:  [4 strips * 128 p, 8 ci * 512 col]   (8KB lines)
    #   wqk: [4 mt * 128 p, 8 ci * 128 col]       (2KB lines)
    #   wv:  [128 p, 8 ci * 256 col]              (4KB lines)
    #   wp:  [128 p, 2 pair * 1024 col]           (4KB lines)
    xT_d = nc.dram_tensor("xT", [NSTRIP * 128, NCT * TQ], bf16,
                          kind="ExternalInput").ap()
    wqk_d = nc.dram_tensor("wqk", [4 * 128, NCT * 128], bf16,
                           kind="ExternalInput").ap()
    wv_d = nc.dram_tensor("wv", [128, NCT * 256], bf16, kind="ExternalInput").ap()
    wp_d = nc.dram_tensor("wp", [128, 2 * C], bf16, kind="ExternalInput").ap()
    if with_bias:
        bqk_d = nc.dram_tensor("bqk", [1, 512], bf16, kind="ExternalInput").ap()
        bv_d = nc.dram_tensor("bv", [1, 256], bf16, kind="ExternalInput").ap()
    out_d = nc.dram_tensor("outT", [C, T], bf16, kind="ExternalOutput").ap()

    with tile.TileContext(nc) as tc:
        with (
            tc.tile_pool(name="persist", bufs=1) as pp,
            tc.tile_pool(name="pP", bufs=10) as pP,
            tc.tile_pool(name="rrow", bufs=6) as pRR,
            tc.tile_pool(name="bcsb", bufs=6) as pBC,
            tc.tile_pool(name="ob", bufs=4) as pO,
            tc.tile_pool(name="psum", bufs=1, space="PSUM") as psp,
        ):
            # ---- persistent SBUF tensors (single tiles => single DMAs) --
            # xA strip-major: [p, strip, ci, col]
            xA = pp.tile([128, NSTRIP * NCT * TQ], bf16, tag="xA", name="xA")
            # wqkA mt-major: [p, mt, ci, col]
            wqkA = pp.tile([128, 4 * NCT * 128], bf16, tag="wqkA", name="wqkA")
            wvA = pp.tile([128, NCT * 256], bf16, tag="wvA", name="wvA")
            wp = pp.tile([128, 2 * C], bf16, tag="wp", name="wp")
            QTp = [pp.tile([128, T], bf16, tag=f"QT{p}", name=f"QT{p}")
                   for p in range(2)]
            KTp = [pp.tile([128, T], bf16, tag=f"KT{p}", name=f"KT{p}")
                   for p in range(2)]
            V2 = pp.tile([128, 2 * NKT * VST], bf16, tag="V2", name="V2")
            yT2 = [pp.tile([128, T], bf16, tag=f"yT{p}", name=f"yT{p}")
                   for p in range(2)]
            tri2 = pp.tile([128, 256], bf16, tag="tri2", name="tri2")
            ones64 = pp.tile([1, 64], bf16, tag="ones64", name="ones64")
            wtile = pp.tile([128, 128], bf16, tag="wtile", name="wtile")
            # tail-chain transpose scratch: denominator row -> 32x32-block
            # transposed -> strided reciprocal -> transposed back
            dcol = pp.tile([32, TQ], f32, tag="dcol", name="dcol")
            dT32 = pp.tile([32, TQ], f32, tag="dT32", name="dT32")
            recT = pp.tile([32, TQ], bf16, tag="recT", name="recT")
            drow2 = pp.tile([32, TQ], bf16, tag="drow2", name="drow2")
            if with_bias:
                bqk = pp.tile([1, 512], bf16, tag="bqk", name="bqk")
                bv = pp.tile([1, 256], bf16, tag="bv", name="bv")
                ones_row = pp.tile([1, 512], bf16, tag="ones", name="ones")

            def xa(ci, strip):
                base = (strip * NCT + ci) * TQ
                return xA[:, base:base + TQ]

            def wqka(ci, mt):
                base = (mt * NCT + ci) * 128
                return wqkA[:, base:base + 128]

            def wva(ci):
                return wvA[:, ci * 256:(ci + 1) * 256]

            # ---- input DMAs: few, large lines, ordered by first use ----
            def x_strip_dma(eng, strip):
                eng.dma_start(
                    xA[:, strip * NCT * TQ:(strip + 1) * NCT * TQ],
                    xT_d[strip * 128:(strip + 1) * 128, :])

            def wqk_mt_dma(eng, mt):
                eng.dma_start(
                    wqkA[:, mt * NCT * 128:(mt + 1) * NCT * 128],
                    wqk_d[mt * 128:(mt + 1) * 128, :])

            # P0 needs wqk mt 0/2 + x strip 0 (qk) + wv (v steps 0-3).
            wqk_mt_dma(nc.sync, 0)
            wqk_mt_dma(nc.scalar, 2)
            x_strip_dma(nc.sync, 0)
            wqk_mt_dma(nc.scalar, 1)
            nc.scalar.dma_start(wvA[:], wv_d[:, :])
            x_strip_dma(nc.sync, 1)
            wqk_mt_dma(nc.scalar, 3)
            x_strip_dma(nc.scalar, 2)
            x_strip_dma(nc.sync, 3)
            nc.scalar.dma_start(wp[:], wp_d[:, :])

            # ---- PE warm-up during the DMA prologue --------------------
            # the scratch tile is memset by the (idle) vector engine so the
            # warm-ups have no gpsimd dependency and start immediately.
            nc.vector.memset(wtile[:], 0.015625)
            warm = psp.tile([128, 128], f32, tag="S", bufs=2, name="warm")
            for _ in range(46):
                nc.tensor.matmul(warm[:], lhsT=wtile[:], rhs=wtile[:],
                                 start=True, stop=True)

            def keep_warm(n=1, cols=128):
                """Cheap dummy matmuls to keep the PE HAM clock-gate open.
                They write (a prefix of) the current 'S'-tag psum slot, so
                their WAR deps mirror the real S steps' and never add waits
                beyond what the next S matmul already has."""
                wps = psp.tile([128, cols], f32, tag="S", bufs=2, name="warm2")
                for _ in range(n):
                    nc.tensor.matmul(wps[:], lhsT=wtile[:], rhs=wtile[:, 0:cols],
                                     start=True, stop=True)

            # ---- constants ---------------------------------------------
            # tri2[i, 256] = two side-by-side copies of tri[i, j] = (i <= j)
            nc.gpsimd.memset(tri2[:], 1.0)
            for hh in range(2):
                nc.gpsimd.affine_select(
                    out=tri2[:, hh * 128:(hh + 1) * 128],
                    in_=tri2[:, hh * 128:(hh + 1) * 128],
                    compare_op=mybir.AluOpType.is_ge,
                    fill=0.0, base=0, pattern=[[1, 128]], channel_multiplier=-1)
            if with_bias:
                nc.sync.dma_start(bqk[:], bqk_d[:, :])
                nc.sync.dma_start(bv[:], bv_d[:, :])
                nc.gpsimd.memset(ones_row[:], 1.0)
            # ones columns in V2 (denominator accumulators), one strided memset
            v2v = V2[:].rearrange("q (p kt h e) -> q p kt h e", p=2, kt=NKT, h=2)
            nc.gpsimd.memset(v2v[:, :, :, :, 64:65], 1.0)
            nc.gpsimd.memset(ones64[:], 1.0)
            nc.vector.memset(dcol[:], 1.0)
            nc.vector.memset(recT[:], 1.0)

            nbias = 1 if with_bias else 0

            # ---- QKV projection steps ----------------------------------
            def qk_step(pair, mt, qt):
                def f():
                    ps = psp.tile([128, TQ], f32, tag="big", bufs=2, name="psA")
                    for ci in range(NCT):
                        nc.tensor.matmul(
                            ps[:],
                            lhsT=wqka(ci, mt),
                            rhs=xa(ci, qt),
                            start=(ci == 0), stop=(ci == NCT + nbias - 1))
                    if with_bias:
                        nc.tensor.matmul(
                            ps[:], lhsT=bqk[0:1, mt * 128:(mt + 1) * 128],
                            rhs=ones_row[0:1, 0:TQ], start=False, stop=True)
                    dst = QTp[pair] if mt < 2 else KTp[pair]
                    nc.vector.tensor_copy(dst[:, qt * TQ:(qt + 1) * TQ], ps[:])
                return f

            def v_step(kt):
                def f():
                    psv = psp.tile([128, 256], f32, tag="big", bufs=2, name="psVt")
                    strip, sub = kt // 4, kt % 4
                    for ci in range(NCT):
                        nc.tensor.matmul(
                            psv[:],
                            lhsT=xa(ci, strip)[:, sub * 128:(sub + 1) * 128],
                            rhs=wva(ci),
                            start=(ci == 0), stop=(ci == NCT + nbias - 1))
                    if with_bias:
                        nc.tensor.matmul(
                            psv[:], lhsT=ones_row[0:1, 0:128], rhs=bv[0:1, :],
                            start=False, stop=True)
                    # [128, 2p, 2h, 64] -> strided V2 slots in one copy
                    s4 = psv[:].rearrange("q (p h b) -> q p h b", p=2, h=2)
                    d4 = v2v[:, :, kt, :, 0:64]
                    nc.vector.tensor_copy(d4, s4)
                return f

            def A_qk(pair, qt):
                return [qk_step(pair, mt, qt) for mt in (pair, 2 + pair)]

            def A_v(qt):
                return [v_step(kt) for kt in range(4 * qt, 4 * qt + 4)]

            # ---- softmax normalization chains --------------------------
            def chain_step(pair, qt, hh, psy):
                """1/denominator, broadcast over 64 partitions, times psy:
                transpose the denominator row to partitions with a DMA
                round-trip, bf16 reciprocal, gpsimd partition_broadcast."""
                def f():
                    ph = psy[hh]
                    drow = pRR.tile([1, TQ], f32, tag="rr", name="drow")
                    nc.vector.tensor_copy(drow[:], ph[64:65, :])
                    rec4 = pRR.tile([128, 4], f32, tag="r4", name="rec4")
                    nc.sync.dma_start(rec4[:, :], drow[0:1, :])
                    rec4b = pRR.tile([128, 4], bf16, tag="r4b", name="rec4b")
                    with nc.allow_low_precision("softmax recip in bf16"):
                        nc.vector.reciprocal(rec4b[:], rec4[:])
                    rrow = pRR.tile([1, TQ], bf16, tag="rrb", name="rrow")
                    nc.sync.dma_start(rrow[0:1, :], rec4b[:, :])
                    bcs = pBC.tile([64, TQ], bf16, tag="bc", name="bcs")
                    nc.gpsimd.partition_broadcast(bcs[:], rrow[0:1, :])
                    nc.vector.tensor_mul(
                        yT2[pair][hh * 64:(hh + 1) * 64, qt * TQ:(qt + 1) * TQ],
                        ph[0:64, :], bcs[:])
                return f

            # ---- fused S + PV strip ------------------------------------
            # Each k-tile step: row-tiled concurrent S matmul pair (the two
            # heads on PE rows 0-63 / 64-127), one exp, and - lagged by a few
            # steps so the previous strip's psum frees - the two heads' psy
            # accumulation matmuls. Chains are returned separately.
            def strip_steps(pair, qt, warm_every=0):
                nk = 4 * (qt + 1)
                P_store = {}
                psy = [None, None]
                LAG = 5

                def s_step(kt):
                    def f():
                        ps = psp.tile([128, 2 * TQ], f32, tag="S", bufs=2, name="psS")
                        m = kt - 4 * qt
                        off = max(0, m) * 128
                        for hh in range(2):
                            nc.tensor.matmul(
                                ps[:, hh * TQ + off:(hh + 1) * TQ],
                                lhsT=KTp[pair][hh * 64:(hh + 1) * 64,
                                               kt * 128:(kt + 1) * 128],
                                rhs=QTp[pair][hh * 64:(hh + 1) * 64,
                                              qt * TQ + off:(qt + 1) * TQ],
                                start=True, stop=True)
                        Pt = pP.tile([128, 2 * TQ], bf16, tag="P", bufs=10, name="Pt")
                        if m < 0:    # one exp across both heads' banks
                            nc.scalar.activation(Pt[:, :], ps[:, :], Exp, scale=0.125)
                        else:        # one strided exp covering both heads
                            pse = ps[:].rearrange("p (h w) -> p h w", h=2)[:, :, off:TQ]
                            pte = Pt[:].rearrange("p (h w) -> p h w", h=2)[:, :, off:TQ]
                            nc.scalar.activation(pte, pse, Exp, scale=0.125)
                            # diagonal block: keep k <= q only, both heads at once
                            ptm = Pt[:].rearrange("p (h w) -> p h w", h=2)[:, :, off:off + 128]
                            nc.vector.tensor_mul(
                                ptm, ptm,
                                tri2[:].rearrange("p (h w) -> p h w", h=2))
                        P_store[kt] = Pt
                    return f

                def psy_step(kt):
                    def f():
                        off = max(0, kt - 4 * qt) * 128
                        if kt == 0:
                            for hh in range(2):
                                psy[hh] = psp.tile([65, TQ], f32, tag="pv",
                                                   bufs=2, name="psy")
                        Pt = P_store.pop(kt)
                        for hh in range(2):
                            nc.tensor.matmul(
                                psy[hh][:, off:TQ],
                                lhsT=v2v[:, pair, kt, hh, 0:65],
                                rhs=Pt[:, hh * TQ + off:(hh + 1) * TQ],
                                start=(kt == 0), stop=(kt == nk - 1))
                    return f

                steps = []
                emitted = 0
                for kt in range(nk):
                    steps.append(s_step(kt))
                    if warm_every and kt % warm_every == warm_every - 1:
                        steps.append(lambda: keep_warm(1))
                    backlog = min(LAG, nk - 1 - kt)
                    while kt + 1 - emitted > backlog:
                        steps.append(psy_step(emitted))
                        emitted += 1
                chains = [chain_step(pair, qt, hh, psy) for hh in range(2)]
                return steps, chains, psy

            # ---- output projection -------------------------------------
            def co_mm(pso, pair, co, qt):
                nc.tensor.matmul(
                    pso[:],
                    lhsT=wp[:, pair * C + co * 128:pair * C + (co + 1) * 128],
                    rhs=yT2[pair][:, qt * TQ:(qt + 1) * TQ],
                    start=(pair == 0), stop=(pair == 1))

            def co_out(pso, co, qt, eng=None, scopy=False):
                ob = pO.tile([128, TQ], bf16, tag="ob", name="ob")
                if scopy:
                    nc.scalar.copy(ob[:], pso[:])
                else:
                    nc.vector.tensor_copy(ob[:], pso[:])
                (eng or nc.sync).dma_start(
                    out_d[co * 128:(co + 1) * 128, qt * TQ:(qt + 1) * TQ],
                    ob[:])

            def PJ_steps(qt):
                def co_step(co):
                    def f():
                        pso = psp.tile([128, TQ], f32, tag="big", bufs=2, name="psO")
                        for pair in range(2):
                            co_mm(pso, pair, co, qt)
                        co_out(pso, co, qt)
                    return f

                return [co_step(co) for co in range(8)]

            def PJ_final(qt, psy):
                """Final-strip projection, interleaved with the last two
                normalization chains. Those chains avoid DMA round-trips and
                the gpsimd launch lag entirely: the denominator row is
                partition-transposed with the DVE 32x32 block transpose, the
                bf16 reciprocal runs on a strided view, a second block
                transpose restores the row, and a K=1 PE matmul broadcasts it
                over 64 partitions. PE instructions are emitted in
                expected-readiness order (the engine queue is strict FIFO -
                a blocked matmul stalls everything behind it). Output DMAs
                alternate HWDGE queues."""
                pend = {}
                bcp = [None, None]

                def half_mm(pso, co, hh, start, stop):
                    sl = slice(hh * 64, (hh + 1) * 64)
                    nc.tensor.matmul(
                        pso[:],
                        lhsT=wp[sl, C + co * 128:C + (co + 1) * 128],
                        rhs=yT2[1][sl, qt * TQ:(qt + 1) * TQ],
                        start=start, stop=stop)

                def tr_pre(hh):
                    def f():
                        nc.scalar.copy(dcol[0:1, :], psy[hh][64:65, :])
                    return f

                def tr_mid(hh):
                    def f():
                        # dT32[k, 32j] = dcol[0, 32j + k]
                        nc.vector.transpose(dT32[:], dcol[:])
                        tsrc = dT32[:].rearrange("p (j c) -> p j c", c=32)
                        tdst = recT[:].rearrange("p (j c) -> p j c", c=32)
                        with nc.allow_low_precision("softmax recip in bf16"):
                            nc.vector.reciprocal(tdst[:, :, 0:1],
                                                 tsrc[:, :, 0:1])
                        # drow2[0, 32j + k] = recT[k, 32j]
                        nc.vector.transpose(drow2[:], recT[:])
                    return f

                def tr_bcast(hh):
                    def f():
                        bcp[hh] = psp.tile([64, TQ], f32, tag="S", bufs=2,
                                           name="bcp")
                        nc.tensor.matmul(bcp[hh][:], lhsT=ones64[:],
                                         rhs=drow2[0:1, :], start=True,
                                         stop=True)
                    return f

                def tr_post(hh):
                    def f():
                        bcs = pBC.tile([64, TQ], bf16, tag="bc", name="bcs")
                        nc.scalar.copy(bcs[:], bcp[hh][:])
                        nc.vector.tensor_mul(
                            yT2[1][hh * 64:(hh + 1) * 64,
                                   qt * TQ:(qt + 1) * TQ],
                            psy[hh][0:64, :], bcs[:])
                    return f

                def open_step(co, tag):
                    def f():
                        pso = psp.tile([128, TQ], f32, tag=tag, bufs=2, name="psO")
                        co_mm(pso, 0, co, qt)
                        pend[co] = pso
                    return f

                def half0_step(co):
                    def f():
                        half_mm(pend[co], co, 0, start=False, stop=False)
                    return f

                def close_step(co):
                    def f():
                        pso = pend.pop(co)
                        half_mm(pso, co, 1, start=False, stop=True)
                        co_out(pso, co, qt, nc.scalar if co % 2 else nc.sync,
                               scopy=bool(co % 2))
                    return f

                def full_step(co):
                    def f():
                        pso = psp.tile([128, TQ], f32, tag="big", bufs=2, name="psO")
                        for pair in range(2):
                            co_mm(pso, pair, co, qt)
                        co_out(pso, co, qt, nc.scalar if co % 2 else nc.sync,
                               scopy=bool(co % 2))
                    return f

                seq = [tr_pre(0), tr_mid(0), tr_pre(1),
                       open_step(0, "big"), open_step(1, "big"),
                       lambda: keep_warm(4),
                       tr_bcast(0), tr_mid(1), tr_bcast(1),
                       tr_post(0),
                       lambda: keep_warm(2),
                       tr_post(1),
                       half0_step(0), half0_step(1),
                       open_step(2, "pv"), open_step(3, "pv"),
                       half0_step(2), half0_step(3),
                       close_step(0), close_step(1),
                       close_step(2), close_step(3),
                       full_step(4), full_step(5),
                       full_step(6), full_step(7)]
                return seq

            def weave(s_list, others):
                """Interleave `others` proportionally between strip steps."""
                if not s_list:
                    for f in others:
                        f()
                    return
                r = len(others) / len(s_list)
                acc, oi = 0.5, 0
                for f in s_list:
                    f()
                    acc += r
                    while acc >= 1.0 and oi < len(others):
                        others[oi]()
                        oi += 1
                        acc -= 1.0
                while oi < len(others):
                    others[oi]()
                    oi += 1

            # ---- phase schedule ----------------------------------------
            # strips in order (0,0),(0,1),(1,0),(0,2),(1,1),(0,3),(1,2),(1,3)
            # with QKV step groups placed one phase before their consumers
            # and projections as late fillers.
            for f in A_qk(0, 0) + A_v(0):
                f()

            s00, c00, _ = strip_steps(0, 0)
            s01, c01, _ = strip_steps(0, 1)
            s10, c10, _ = strip_steps(1, 0)
            s02, c02, _ = strip_steps(0, 2, warm_every=4)
            s11, c11, _ = strip_steps(1, 1, warm_every=4)
            s03, c03, _ = strip_steps(0, 3, warm_every=2)
            s12, c12, _ = strip_steps(1, 2, warm_every=2)
            s13, _c13, psy13 = strip_steps(1, 3, warm_every=2)

            pj0 = PJ_steps(0)
            pj1 = PJ_steps(1)
            pj2 = PJ_steps(2)

            weave(s00 + c00, A_qk(0, 1) + A_qk(1, 0) + A_v(1))
            weave(s01 + c01, A_qk(0, 2) + A_v(2))
            weave(s10 + c10, A_qk(1, 1) + A_qk(0, 3))
            weave(s02 + c02, A_v(3) + pj0[:4])
            weave(s11 + c11, A_qk(1, 2) + pj0[4:])
            weave(s03 + c03, A_qk(1, 3) + pj1[:4])
            weave(s12 + c12, pj1[4:])
            weave(s13, pj2)
            weave([], PJ_final(3, psy13))

    nc.compile()
    return nc


def _get_nc(with_bias: bool):
    key = ("nc", with_bias)
    if key not in _CACHE:
        _ensure_runtime()
        _CACHE[key] = _build(with_bias)
    return _CACHE[key]


def _shard_inputs(x, w_qkv, b_qkv, w_proj, with_bias):
    """Build the 8 per-core input maps (bf16), repacked for large DMA lines."""
    in_maps = []
    for core in range(N_CORES):
        b, g = core // G, core % G
        hs = [g * HPG + i for i in range(HPG)]
        q_cols = [w_qkv[:, h * DH:(h + 1) * DH] for h in hs]
        k_cols = [w_qkv[:, C + h * DH: C + (h + 1) * DH] for h in hs]
        v_cols = [w_qkv[:, 2 * C + h * DH: 2 * C + (h + 1) * DH] for h in hs]

        # x: [C, T] -> strip-major [strip*128, ci*512]
        xT = np.ascontiguousarray(x[b].T)              # [C, T]
        xs = xT.reshape(NCT, 128, NSTRIP, TQ)          # [ci, p, strip, col]
        xs = xs.transpose(2, 1, 0, 3).reshape(NSTRIP * 128, NCT * TQ)

        # wqk: [C, 512] (q01 q23 k01 k23) -> mt-major [mt*128, ci*128]
        wqk = np.concatenate(q_cols + k_cols, axis=1)  # [C, 512]
        ws = wqk.reshape(NCT, 128, 4, 128)             # [ci, p, mt, col]
        ws = ws.transpose(2, 1, 0, 3).reshape(4 * 128, NCT * 128)

        # wv: [C, 256] -> [128, ci*256]
        wv = np.concatenate(v_cols, axis=1)            # [C, 256]
        wvs = wv.reshape(NCT, 128, 256).transpose(1, 0, 2).reshape(128, NCT * 256)

        # wp: [256, C] -> [128, pair*1024]
        wpm = np.concatenate(
            [w_proj[h * DH:(h + 1) * DH, :] for h in hs], axis=0)  # [256, C]
        wps = wpm.reshape(2, 128, C).transpose(1, 0, 2).reshape(128, 2 * C)

        m = {
            "xT": np.ascontiguousarray(xs).astype(BF),
            "wqk": np.ascontiguousarray(ws).astype(BF),
            "wv": np.ascontiguousarray(wvs).astype(BF),
            "wp": np.ascontiguousarray(wps).astype(BF),
        }
        if with_bias:
            bq = [b_qkv[h * DH:(h + 1) * DH] for h in hs]
            bk = [b_qkv[C + h * DH: C + (h + 1) * DH] for h in hs]
            bvs = [b_qkv[2 * C + h * DH: 2 * C + (h + 1) * DH] for h in hs]
            m["bqk"] = np.concatenate(bq + bk)[None, :].astype(BF)
            m["bv"] = np.concatenate(bvs)[None, :].astype(BF)
        in_maps.append(m)
    return in_maps


def run_on_device(x, w_qkv, b_qkv, w_proj, b_proj, trace=False, trace_kwargs=None):
    """Returns (output [B,T,C] float32, BassKernelResults)."""
    x = np.asarray(x, np.float32)
    w_qkv = np.asarray(w_qkv, np.float32)
    b_qkv = np.asarray(b_qkv, np.float32)
    w_proj = np.asarray(w_proj, np.float32)
    b_proj = np.asarray(b_proj, np.float32)

    with_bias = bool(np.any(b_qkv))
    nc = _get_nc(with_bias)
    in_maps = _shard_inputs(x, w_qkv, b_qkv, w_proj, with_bias)

    from concourse.bass_utils import run_bass_kernel_spmd
    res = run_bass_kernel_spmd(nc, in_maps, core_ids=list(range(N_CORES)),
                               trace=trace, **(trace_kwargs or {}))

    out = np.zeros((B, T, C), np.float64)
    for core in range(N_CORES):
        b = core // G
        out[b] += res.results[core]["outT"].T.astype(np.float64)
    out += b_proj.astype(np.float64)[None, None, :]
    return out.astype(np.float32), res


def kernel(x, w_qkv, b_qkv, w_proj, b_proj):
    out, _ = run_on_device(x, w_qkv, b_qkv, w_proj, b_proj)
    return out
